# revision 6
# baseline (speedup 1.0000x reference)
"""Trainium2 Bass kernel for a 3-layer MLP actor network with a
positive-softmax-tanh head, data-parallel over 8 NeuronCores.

Network (per row of the batch):
    x1 = relu(state @ W1.T + b1)        # 512 -> 400
    x2 = relu(x1 @ W2.T + b2)           # 400 -> 300
    y  = x2 @ W3.T + b3                 # 300 -> 64
    head = tanh(y[:63]); last = sigmoid(y[63])
    s = sum(relu(head)); head = where(head > 0, head / s, head)
    out = concat(head, last)

Strategy:
  - Pure data parallel: each of 8 cores gets 16384 rows of the batch.
  - All matmuls run in a "transposed" layout (features on partitions,
    batch on the free dim) so each layer's bias is a per-partition
    operand fused into the PSUM->SBUF relu pass.
  - The host pre-transposes/casts state to bf16 tiles so all device DMAs
    are natural contiguous loads (no on-device input transpose).
  - Output-feature chunks are 128 wide (last chunk ragged) so stationary
    weight loads qualify for fast-weight-load.
  - Ragged tails use PE array tiling (tile_position) so the A/B tiles of
    a pair run in disjoint 32-wide column/row groups concurrently:
      * L1 m-tail (M=16): A at cols 0-15, B at cols 32-47 -> B's x1 tail
        lands on partitions 32-47, which row-packs L2's ragged K=16
        contraction chunk (A rows 0-15, B rows 32-47, concurrent).
      * L2 m-tail (M=44): A cols 0-43, B cols 64-107.
      * L3 (M=64): A cols 0-63, B cols 64-127.
  - The head epilogue transposes y back to [batch, 64] with the DMA XBAR
    (dma_start_transpose, bf16) instead of burning PE passes; in natural
    layout the positive-sum `s` is a per-partition scalar: DVE max+accum
    computes it, DVE reciprocal inverts it, and one fused
    scalar_tensor_tensor applies
    out = tanh + relu(tanh) * (1/s - 1)  ==  where(tanh>0, tanh/s, tanh).
  - DMA instructions are the scarce resource after the PE (each costs
    ~0.6us of shared HWDGE descriptor-generation): state tiles load with
    ONE instruction per tile, the batch order within a tile is permuted
    host-side (n = 128*j2+p holds original row 4p+j2) so each partition's
    8 output rows are DRAM-contiguous (1KB stores), and output stores go
    through the idle GpSimd SWDGE queue. Prefetch loads (SP queue) are
    separated from late-dependency DMAs (transposes on the ACT queue,
    stores on GpSimd) so a waiting DMA never head-of-line blocks the
    prefetch stream.
"""

import contextlib
import sys

for _p in ("/opt/trn_rl_repo",):
    if _p not in sys.path:
        sys.path.insert(0, _p)

import numpy as np
import ml_dtypes

import concourse.bass as bass
import concourse.tile as tile
from concourse.tile_rust import add_dep_helper
from concourse import bacc, mybir
from concourse.bass_utils import run_bass_kernel_spmd

N_CORES = 8
B = 131072
D, H1, H2, A = 512, 400, 300, 64
BC = B // N_CORES          # rows per core = 16384
NTILE = 512                # batch columns processed per tile
NT = BC // NTILE           # 32 tiles per core
SUB = NTILE // 128         # 4 sub-tiles of 128 batch rows per tile


def chunks(n, size=128):
    """[(offset, width), ...] covering n in `size`-wide pieces."""
    return [(o, min(size, n - o)) for o in range(0, n, size)]

CH_D = chunks(D)       # layer-1 contraction: 4 x 128
CH_H1 = chunks(H1)     # 128,128,128,16
CH_H2 = chunks(H2)     # 128,128,44
KC1, MC1 = len(CH_D), len(CH_H1)
KC2, MC2 = len(CH_H1), len(CH_H2)
KC3 = len(CH_H2)

AF = mybir.ActivationFunctionType
OP = mybir.AluOpType
BF16 = mybir.dt.bfloat16
F32 = mybir.dt.float32


def build_kernel(nt=NT, repeat=1):
    nc = bacc.Bacc(
        "TRN2", debug=False, target_bir_lowering=False, num_devices=N_CORES
    )

    xt = nc.dram_tensor("xt", [nt, 128, KC1, NTILE], BF16, kind="ExternalInput")
    w1t = nc.dram_tensor("w1t", [KC1, 128, H1], BF16, kind="ExternalInput")
    # w2t/w3t zero-padded to a whole number of 128-row chunks so each loads
    # with a single rearranging DMA
    w2t = nc.dram_tensor("w2t", [KC2 * 128, H2], BF16, kind="ExternalInput")
    w3t = nc.dram_tensor("w3t", [KC3 * 128, A], BF16, kind="ExternalInput")
    # all biases in one [128, MC1+MC2+1] tensor (cols: b1 chunks, b2
    # chunks, b3)
    bb = nc.dram_tensor("bb", [128, MC1 + MC2 + 1], F32, kind="ExternalInput")
    out = nc.dram_tensor("out", [nt * NTILE, A], F32, kind="ExternalOutput")

    with tile.TileContext(nc) as tc:
        with (
            tc.tile_pool(name="consts", bufs=1) as consts,
            tc.tile_pool(name="state", bufs=6) as state_pool,
            tc.tile_pool(name="acts", bufs=2) as acts,
            tc.tile_pool(name="epi", bufs=2) as epi,
            tc.tile_pool(name="ps_x1", bufs=4, space="PSUM") as ps_x1,
            tc.tile_pool(name="ps_x2", bufs=3, space="PSUM") as ps_x2,
            tc.tile_pool(name="ps_y", bufs=1, space="PSUM") as ps_y,
        ):
            # consts go through the ACT-engine HWDGE queue so the SP queue
            # can start streaming state tiles immediately
            w1_sb = consts.tile([128, KC1, H1], BF16)
            nc.scalar.dma_start(out=w1_sb, in_=w1t[:].rearrange("k p c -> p k c"))
            w2_sb = consts.tile([128, KC2, H2], BF16)
            nc.scalar.dma_start(
                out=w2_sb, in_=w2t[:].rearrange("(k p) c -> p k c", p=128)
            )
            # duplicate the ragged K=16 chunk at partition base 32 for the
            # row-packed B-side matmuls
            nc.scalar.dma_start(out=w2_sb[32:48, 3, :], in_=w2t[384:400, :])
            w3_sb = consts.tile([128, KC3, A], BF16)
            nc.scalar.dma_start(
                out=w3_sb, in_=w3t[:].rearrange("(k p) c -> p k c", p=128)
            )
            # duplicate the ragged K=44 chunk at partition base 64
            nc.scalar.dma_start(out=w3_sb[64:108, 2, :], in_=w3t[256:300, :])
            bb_sb = consts.tile([128, MC1 + MC2 + 1], F32)
            nc.scalar.dma_start(out=bb_sb, in_=bb[:])
            b1_sb = bb_sb[:, 0:MC1]
            b2_sb = bb_sb[:, MC1 : MC1 + MC2]
            b3_sb = bb_sb[:, MC1 + MC2 : MC1 + MC2 + 1]

            # Tile inserts a full engine drain at the For_i back-edge
            # (~20us of pipeline refill per iteration). Unroll the body 2x
            # so the drain is paid half as often.
            unroll = 2 if repeat > 1 and repeat % 2 == 0 else 1
            n_iter = repeat // unroll
            rep_ctx = (
                tc.For_i(0, n_iter, 1) if n_iter > 1 else contextlib.nullcontext()
            )
            with rep_ctx:
                for _u in range(unroll):
                    _tile_loop(nc, tc, nt, locals(), cold_start=(_u == 0))

    nc.compile()
    _dedupe_ldweights(nc)
    return nc


def _dedupe_ldweights(nc):
    """Remove back-to-back redundant PE Ldweights (identical stationary AP
    and tile position, no waits) — the paired A/B matmuls share each
    stationary, so every second load is a no-op the sequencer still pays
    for. Transpose matmuls self-load and clobber the array, resetting the
    tracked signature."""
    import bass_rust

    PE = mybir.EngineType.PE
    n_del = 0
    for fn in nc.m.functions:
        for blk in fn.blocks:
            insts = list(blk.instructions)
            last_sig = None
            keep = []
            for ins in insts:
                if getattr(ins, "engine", None) != PE:
                    keep.append(ins)
                    continue
                tn = type(ins).__name__
                if tn == "InstLdweights":
                    try:
                        ap = ins.ins[0]
                        sig = (
                            str(ap.memref),
                            int(ap.offset),
                            str(ap.ap),
                            str(ins.tile_position),
                            str(ins.perf_mode),
                        )
                    except Exception:
                        sig = None
                    if (
                        sig is not None
                        and sig == last_sig
                        and not ins.has_wait()
                        and not ins.has_update()
                    ):
                        n_del += 1
                        continue
                    last_sig = sig
                    keep.append(ins)
                elif tn == "InstMatmult":
                    if ins.is_transpose:
                        last_sig = None
                    keep.append(ins)
                else:
                    keep.append(ins)
            if len(keep) != len(insts):
                blk.instructions = keep
    return n_del


def _tile_loop(nc, tc, nt, env, cold_start=True):
    xt, out = env["xt"], env["out"]
    w1_sb, w2_sb, w3_sb = env["w1_sb"], env["w2_sb"], env["w3_sb"]
    b1_sb, b2_sb, b3_sb = env["b1_sb"], env["b2_sb"], env["b3_sb"]
    state_pool, acts, epi = env["state_pool"], env["acts"], env["epi"]
    ps_x1, ps_x2, ps_y = env["ps_x1"], env["ps_x2"], env["ps_y"]
    assert nt % 2 == 0
    # A/B partition bases for the ragged ("tail") chunks. The B side of the
    # pair runs its tail matmuls in a disjoint PE array quadrant so both
    # tiles' tails execute concurrently (tile_position packing).
    L1_TAIL_B = 32    # L1 m3 (M=16): A cols 0-15, B cols 32-47
    L2_TAIL_B = 64    # L2 m2 (M=44): A cols 0-43, B cols 64-107
    Y_B = 64          # L3 (M=64): A cols 0-63, B cols 64-127
    n_pairs = nt // 2
    GRP = 1   # pairs per epilogue group
    yb_q = None
    for jp in range(n_pairs):
        g = jp % GRP
        pair = (2 * jp, 2 * jp + 1)
        sts = []
        for pi, j in enumerate(pair):
            st = state_pool.tile(
                [128, KC1, NTILE], BF16, tag=f"st{pi}", name=f"st{pi}"
            )
            # at the iteration start (cold refill after the For_i drain)
            # split the two loads across both HWDGE queues; mid-stream,
            # keep everything on SP so prefetch backpressure never blocks
            # the ACT queue
            q = nc.scalar if (cold_start and jp == 0 and pi == 1) else nc.sync
            q.dma_start(out=st, in_=xt[j])
            sts.append(st)

        # Layer 1. Full-width m chunks use one PSUM group per (m, side);
        # the M=16 tail is col-packed: A at cols 0-15, B at cols 32-47 of
        # one shared PSUM tile (concurrent in disjoint array col groups).
        x1s = [
            acts.tile([128, KC2, NTILE], BF16, tag=f"x1{pi}", name=f"x1{pi}")
            for pi in range(2)
        ]
        for m, (mo, mw) in enumerate(CH_H1[:-1]):
            pss = [
                ps_x1.tile([128, NTILE], F32, tag="psx1", name=f"psx1{pi}")
                for pi in range(2)
            ]
            prev = None
            for k in range(KC1):
                for pi in range(2):
                    mm = nc.tensor.matmul(
                        out=pss[pi][0:mw, :],
                        lhsT=w1_sb[:, k, mo : mo + mw],
                        rhs=sts[pi][:, k, :],
                        start=(k == 0),
                        stop=(k == KC1 - 1),
                    )
                    if prev is not None:
                        add_dep_helper(
                            mm.ins, prev.ins, sync=False, reason="pair ldw share"
                        )
                    prev = mm
            for pi in range(2):
                nc.scalar.activation(
                    out=x1s[pi][0:mw, m, :],
                    in_=pss[pi][0:mw, :],
                    func=AF.Relu,
                    bias=b1_sb[0:mw, m : m + 1],
                )
        mo, mw = CH_H1[-1]
        m3 = len(CH_H1) - 1
        bases1 = (0, L1_TAIL_B)
        ps13 = ps_x1.tile([128, NTILE], F32, tag="psx1", name="ps13")
        for k in range(KC1):
            for pi in range(2):
                b = bases1[pi]
                nc.tensor.matmul(
                    out=ps13[b : b + mw, :],
                    lhsT=w1_sb[:, k, mo : mo + mw],
                    rhs=sts[pi][:, k, :],
                    start=(k == 0),
                    stop=(k == KC1 - 1),
                    tile_position=(0, b),
                    skip_group_check=True,
                )
        for pi in range(2):
            b = bases1[pi]
            nc.scalar.activation(
                out=x1s[pi][b : b + mw, m3, :],
                in_=ps13[b : b + mw, :],
                func=AF.Relu,
                bias=b1_sb[b : b + mw, m3 : m3 + 1],
            )

        # Layer 2. x1's K=16 tail lives at base 0 (A) / 32 (B): row-packed,
        # so the pair's ragged-k matmuls run concurrently in disjoint row
        # groups. The M=44 tail is col-packed at base 0 (A) / 64 (B).
        def l2_k_ap(pi, k, weights_m_slice):
            ko, kw = CH_H1[k]
            b = bases1[pi] if k == m3 else 0
            return (
                w2_sb[b : b + kw, k, weights_m_slice],
                x1s[pi][b : b + kw, k, :],
                b,
            )

        x2s = [
            acts.tile([128, KC3, NTILE], BF16, tag=f"x2{pi}", name=f"x2{pi}")
            for pi in range(2)
        ]
        for m, (mo, mw) in enumerate(CH_H2[:-1]):
            pss = [
                ps_x2.tile([128, NTILE], F32, tag="psx2", name=f"psx2{pi}")
                for pi in range(2)
            ]
            prev = None
            for k in range(KC2):
                for pi in range(2):
                    lhsT, rhs, kb = l2_k_ap(pi, k, slice(mo, mo + mw))
                    mm = nc.tensor.matmul(
                        out=pss[pi][0:mw, :],
                        lhsT=lhsT,
                        rhs=rhs,
                        start=(k == 0),
                        stop=(k == KC2 - 1),
                        tile_position=(kb, 0),
                    )
                    if prev is not None and kb == 0:
                        add_dep_helper(
                            mm.ins, prev.ins, sync=False, reason="pair ldw share"
                        )
                    prev = mm
            for pi in range(2):
                nc.vector.tensor_scalar(
                    out=x2s[pi][0:mw, m, :],
                    in0=pss[pi][0:mw, :],
                    scalar1=b2_sb[0:mw, m : m + 1],
                    scalar2=0.0,
                    op0=OP.add,
                    op1=OP.max,
                )
        mo, mw = CH_H2[-1]
        m2 = len(CH_H2) - 1
        bases2 = (0, L2_TAIL_B)
        ps22 = ps_x2.tile([128, NTILE], F32, tag="psx2", name="ps22")
        for k in range(KC2):
            for pi in range(2):
                lhsT, rhs, kb = l2_k_ap(pi, k, slice(mo, mo + mw))
                b = bases2[pi]
                nc.tensor.matmul(
                    out=ps22[b : b + mw, :],
                    lhsT=lhsT,
                    rhs=rhs,
                    start=(k == 0),
                    stop=(k == KC2 - 1),
                    tile_position=(kb, b),
                    skip_group_check=True,
                )
        for pi in range(2):
            b = bases2[pi]
            nc.vector.tensor_scalar(
                out=x2s[pi][b : b + mw, m2, :],
                in0=ps22[b : b + mw, :],
                scalar1=b2_sb[b : b + mw, m2 : m2 + 1],
                scalar2=0.0,
                op0=OP.add,
                op1=OP.max,
            )

        # Layer 3: both sides col-packed into one PSUM bank
        # (A rows 0-63, B rows 64-127).
        y_ps = ps_y.tile([128, NTILE], F32, tag="yy", name="psy")
        for k, (ko, kw) in enumerate(CH_H2):
            for pi in range(2):
                kb = bases2[pi] if k == m2 else 0
                yb_base = (0, Y_B)[pi]
                nc.tensor.matmul(
                    out=y_ps[yb_base : yb_base + A, :],
                    lhsT=w3_sb[kb : kb + kw, k, :],
                    rhs=x2s[pi][kb : kb + kw, k, :],
                    start=(k == 0),
                    stop=(k == KC3 - 1),
                    tile_position=(kb, yb_base),
                    skip_group_check=True,
                )
        if g == 0:
            yb_q = epi.tile([128, GRP, NTILE], BF16, tag="yb", name="yb")
        nc.scalar.activation(
            out=yb_q[:, g, :], in_=y_ps, func=AF.Identity, bias=b3_sb
        )
        if g != GRP - 1:
            continue

        # One DMA-XBAR transpose flips the whole group's y back to batch-
        # major: block index 4*g+j2 of yt_q holds pair g's sub-tile j2,
        # with side A's features in cols 0-63 and B's in 64-127. Frees
        # the PE of transpose passes and amortizes the DMA instruction.
        yt_q = epi.tile([128, GRP * SUB, 128], BF16, tag="yt", name="yt")
        nc.scalar.dma_start_transpose(out=yt_q, in_=yb_q)

        # Epilogue: tanh head, sigmoid last column, positive-sum
        # normalization with per-partition (per-row) scalars.
        out_q = epi.tile([128, 2 * GRP, SUB, A], F32, tag="outp", name="outp")
        for gg in range(GRP):
            yt_sb = yt_q[:, gg * SUB : (gg + 1) * SUB, :]
            s8 = epi.tile([128, 2 * SUB], F32, tag=f"s8{gg}", name=f"s8{gg}")
            rinv8 = epi.tile(
                [128, 2 * SUB], F32, tag=f"rinv8{gg}", name=f"rinv8{gg}"
            )
            # one tanh over both sides' heads: view yt as [p, SUB, 2, 64]
            # (side t at cols 64t..64t+63), take cols 0..62 of each side
            yt4 = yt_sb.rearrange("p s (t c) -> p s t c", t=2)
            tanh_q = epi.tile(
                [128, SUB, 2, A - 1], F32, tag=f"tanh{gg}", name=f"tanh{gg}"
            )
            nc.scalar.activation(
                out=tanh_q, in_=yt4[:, :, :, 0 : A - 1], func=AF.Tanh
            )
            relu_q = epi.tile(
                [128, SUB, 2, A - 1], F32, tag=f"reluh{gg}", name=f"reluh{gg}"
            )
            for pi in range(2):
                for j2 in range(SUB):
                    idx = pi * SUB + j2
                    nc.vector.tensor_scalar(
                        out=relu_q[:, j2, pi, :],
                        in0=tanh_q[:, j2, pi, :],
                        scalar1=0.0,
                        scalar2=None,
                        op0=OP.max,
                        op1=OP.add,
                        accum_out=s8[:, idx : idx + 1],
                    )
            nc.vector.tensor_scalar_max(s8, s8, 1e-30)
            nc.vector.reciprocal(rinv8, s8)
            nc.vector.tensor_scalar_add(rinv8, rinv8, -1.0)
            # one sigmoid covers both sides' last column
            nc.scalar.activation(
                out=out_q[:, 2 * gg : 2 * gg + 2, :, A - 1 : A],
                in_=yt4[:, :, :, A - 1 : A].rearrange("p s t c -> p t s c"),
                func=AF.Sigmoid,
            )
            for pi in range(2):
                for j2 in range(SUB):
                    idx = pi * SUB + j2
                    nc.vector.scalar_tensor_tensor(
                        out=out_q[:, 2 * gg + pi, j2, 0 : A - 1],
                        in0=relu_q[:, j2, pi, :],
                        scalar=rinv8[:, idx : idx + 1],
                        in1=tanh_q[:, j2, pi, :],
                        op0=OP.mult,
                        op1=OP.add,
                    )
        # host packs tile column n=128*j2+p as original row 4p+j2, so
        # partition p's 4 result rows per tile are DRAM-contiguous (1KB);
        # the whole group's tiles store with ONE DMA on the idle GpSimd
        # SWDGE queue to keep HWDGE free
        t0 = (2 * (jp - GRP + 1)) * NTILE
        nc.gpsimd.dma_start(
            out=out[t0 : t0 + 2 * GRP * NTILE, :].rearrange(
                "(t p q) c -> p t q c", t=2 * GRP, p=128
            ),
            in_=out_q,
        )


def prep_core_inputs(state_c, w1, b1, w2, b2, w3, b3, nt=NT):
    """Build the per-core input map from this core's [BC, 512] f32 shard."""
    bf16 = ml_dtypes.bfloat16
    rows = nt * NTILE

    def pad_bias(b, n_chunks, tail_dup_base=None):
        # [n] -> [128, n_chunks] column-per-chunk layout (ragged tail zero);
        # optionally duplicate the ragged tail chunk at a shifted partition
        # base for the B-side packed matmuls
        out = np.zeros((128, n_chunks), np.float32)
        cs = chunks(len(b))
        for m, (mo, mw) in enumerate(cs):
            out[0:mw, m] = b[mo : mo + mw]
        if tail_dup_base is not None:
            mo, mw = cs[-1]
            out[tail_dup_base : tail_dup_base + mw, -1] = b[mo : mo + mw]
        return out

    # xt[j, p, k, n] = state_c[NTILE*j + perm[n], 128*k + p] — contiguous
    # per partition so each tile loads with one DMA. perm places original
    # row 4p+j2 at column n=128*j2+p, which makes each partition's 4
    # output rows DRAM-contiguous after the epilogue's XBAR transpose.
    n = np.arange(NTILE)
    perm = 4 * (n % 128) + n // 128
    xt = np.ascontiguousarray(
        state_c[:rows]
        .reshape(nt, NTILE, KC1, 128)[:, perm]
        .transpose(0, 3, 2, 1)
    ).astype(bf16)

    def pad_rows(w, rows128):
        out = np.zeros((rows128 * 128, w.shape[1]), w.dtype)
        out[: w.shape[0]] = w
        return out

    bb = np.concatenate(
        [
            pad_bias(b1, MC1, tail_dup_base=32),
            pad_bias(b2, MC2, tail_dup_base=64),
            np.concatenate([b3, b3]).reshape(128, 1).astype(np.float32),
        ],
        axis=1,
    )
    return {
        "xt": xt,
        "w1t": np.ascontiguousarray(w1.T.reshape(KC1, 128, H1)).astype(bf16),
        "w2t": np.ascontiguousarray(pad_rows(w2.T, KC2)).astype(bf16),
        "w3t": np.ascontiguousarray(pad_rows(w3.T, KC3)).astype(bf16),
        "bb": np.ascontiguousarray(bb).astype(np.float32),
    }


_NC_CACHE = {}


def _get_nc(nt=NT):
    if nt not in _NC_CACHE:
        _NC_CACHE[nt] = build_kernel(nt)
    return _NC_CACHE[nt]


def run(state, W1, b1, W2, b2, W3, b3, trace=False, **spmd_kwargs):
    state = np.asarray(state, dtype=np.float32)
    nc = _get_nc()
    in_maps = []
    for c in range(N_CORES):
        shard = state[c * BC : (c + 1) * BC]
        in_maps.append(prep_core_inputs(shard, W1, b1, W2, b2, W3, b3))
    res = run_bass_kernel_spmd(
        nc, in_maps, core_ids=list(range(N_CORES)), trace=trace, **spmd_kwargs
    )
    full = np.concatenate([np.asarray(r["out"]) for r in res.results], axis=0)
    return full.astype(np.float32), res


def kernel(state, W1, b1, W2, b2, W3, b3):
    out, _ = run(state, W1, b1, W2, b2, W3, b3, trace=False)
    return out


def make_runner(nc, in_maps):
    """Build a reusable on-device executor with resident inputs.

    Mirrors bass2jax.run_bass_via_pjrt's multi-core path but without
    donation (this kernel writes every output element) so device inputs
    can be reused across calls — per-call cost is dispatch + execution.
    """
    import jax
    from jax.experimental.shard_map import shard_map
    from jax.sharding import Mesh, NamedSharding, PartitionSpec

    from concourse.bass2jax import (
        _bass_exec_p,
        install_neuronx_cc_hook,
        partition_id_tensor,
    )

    install_neuronx_cc_hook()
    n_cores = len(in_maps)
    partition_name = nc.partition_id_tensor.name if nc.partition_id_tensor else None
    in_names, out_names, out_avals, zero_outs = [], [], [], []
    for alloc in nc.m.functions[0].allocations:
        if not isinstance(alloc, mybir.MemoryLocationSet):
            continue
        name = alloc.memorylocations[0].name
        if alloc.kind == "ExternalInput":
            if name != partition_name:
                in_names.append(name)
        elif alloc.kind == "ExternalOutput":
            shape = tuple(alloc.tensor_shape)
            dtype = mybir.dt.np(alloc.dtype)
            out_names.append(name)
            out_avals.append(jax.core.ShapedArray(shape, dtype))
            zero_outs.append(np.zeros(shape, dtype))
    n_params = len(in_names)
    all_in = list(in_names) + list(out_names)
    if partition_name is not None:
        all_in.append(partition_name)

    def _body(*args):
        operands = list(args)
        if partition_name is not None:
            operands.append(partition_id_tensor())
        outs = _bass_exec_p.bind(
            *operands,
            out_avals=tuple(out_avals),
            in_names=tuple(all_in),
            out_names=tuple(out_names),
            lowering_input_output_aliases=(),
            sim_require_finite=True,
            sim_require_nnan=True,
            nc=nc,
        )
        return tuple(outs)

    devices = jax.devices()[:n_cores]
    mesh = Mesh(np.asarray(devices), ("core",))
    in_specs = (PartitionSpec("core"),) * (n_params + len(out_names))
    out_specs = (PartitionSpec("core"),) * len(out_names)
    fn = jax.jit(
        shard_map(
            _body, mesh=mesh, in_specs=in_specs, out_specs=out_specs, check_rep=False
        ),
        keep_unused=True,
    )
    sharding = NamedSharding(mesh, PartitionSpec("core"))
    concat_in = [
        np.concatenate([np.asarray(in_maps[c][nm]) for c in range(n_cores)], axis=0)
        for nm in in_names
    ]
    concat_zero = [
        np.zeros((n_cores * z.shape[0], *z.shape[1:]), z.dtype) for z in zero_outs
    ]
    dev_args = [jax.device_put(a, sharding) for a in concat_in + concat_zero]

    def call():
        outs = fn(*dev_args)
        jax.block_until_ready(outs)
        return outs

    call.fn = fn
    call.dev_args = dev_args
    return call, out_names, out_avals


# revision 7
# speedup vs baseline: 1.0240x; 1.0240x over previous
"""Trainium2 Bass kernel for a 3-layer MLP actor network with a
positive-softmax-tanh head, data-parallel over 8 NeuronCores.

Network (per row of the batch):
    x1 = relu(state @ W1.T + b1)        # 512 -> 400
    x2 = relu(x1 @ W2.T + b2)           # 400 -> 300
    y  = x2 @ W3.T + b3                 # 300 -> 64
    head = tanh(y[:63]); last = sigmoid(y[63])
    s = sum(relu(head)); head = where(head > 0, head / s, head)
    out = concat(head, last)

Strategy:
  - Pure data parallel: each of 8 cores gets 16384 rows of the batch.
  - All matmuls run in a "transposed" layout (features on partitions,
    batch on the free dim) so each layer's bias is a per-partition
    operand fused into the PSUM->SBUF relu pass.
  - The host pre-transposes/casts state to bf16 tiles so all device DMAs
    are natural contiguous loads (no on-device input transpose).
  - Output-feature chunks are 128 wide (last chunk ragged) so stationary
    weight loads qualify for fast-weight-load.
  - Ragged tails use PE array tiling (tile_position) so the A/B tiles of
    a pair run in disjoint 32-wide column/row groups concurrently:
      * L1 m-tail (M=16): A at cols 0-15, B at cols 32-47 -> B's x1 tail
        lands on partitions 32-47, which row-packs L2's ragged K=16
        contraction chunk (A rows 0-15, B rows 32-47, concurrent).
      * L2 m-tail (M=44): A cols 0-43, B cols 64-107.
      * L3 (M=64): A cols 0-63, B cols 64-127.
  - The head epilogue transposes y back to [batch, 64] with the DMA XBAR
    (dma_start_transpose, bf16) instead of burning PE passes; in natural
    layout the positive-sum `s` is a per-partition scalar: DVE max+accum
    computes it, DVE reciprocal inverts it, and one fused
    scalar_tensor_tensor applies
    out = tanh + relu(tanh) * (1/s - 1)  ==  where(tanh>0, tanh/s, tanh).
  - DMA instructions are the scarce resource after the PE (each costs
    ~0.6us of shared HWDGE descriptor-generation): state tiles load with
    ONE instruction per tile, the batch order within a tile is permuted
    host-side (n = 128*j2+p holds original row 4p+j2) so each partition's
    8 output rows are DRAM-contiguous (1KB stores), and output stores go
    through the idle GpSimd SWDGE queue. Prefetch loads (SP queue) are
    separated from late-dependency DMAs (transposes on the ACT queue,
    stores on GpSimd) so a waiting DMA never head-of-line blocks the
    prefetch stream.
"""

import contextlib
import sys

for _p in ("/opt/trn_rl_repo",):
    if _p not in sys.path:
        sys.path.insert(0, _p)

import numpy as np
import ml_dtypes

import concourse.bass as bass
import concourse.tile as tile
from concourse.tile_rust import add_dep_helper
from concourse import bacc, mybir
from concourse.bass_utils import run_bass_kernel_spmd

N_CORES = 8
B = 131072
D, H1, H2, A = 512, 400, 300, 64
BC = B // N_CORES          # rows per core = 16384
NTILE = 512                # batch columns processed per tile
NT = BC // NTILE           # 32 tiles per core
SUB = NTILE // 128         # 4 sub-tiles of 128 batch rows per tile


def chunks(n, size=128):
    """[(offset, width), ...] covering n in `size`-wide pieces."""
    return [(o, min(size, n - o)) for o in range(0, n, size)]

CH_D = chunks(D)       # layer-1 contraction: 4 x 128
CH_H1 = chunks(H1)     # 128,128,128,16
CH_H2 = chunks(H2)     # 128,128,44
KC1, MC1 = len(CH_D), len(CH_H1)
KC2, MC2 = len(CH_H1), len(CH_H2)
KC3 = len(CH_H2)

AF = mybir.ActivationFunctionType
OP = mybir.AluOpType
BF16 = mybir.dt.bfloat16
F32 = mybir.dt.float32


def build_kernel(nt=NT, repeat=1):
    nc = bacc.Bacc(
        "TRN2", debug=False, target_bir_lowering=False, num_devices=N_CORES
    )

    xt = nc.dram_tensor("xt", [nt, 128, KC1, NTILE], BF16, kind="ExternalInput")
    w1t = nc.dram_tensor("w1t", [KC1, 128, H1], BF16, kind="ExternalInput")
    # w2t/w3t zero-padded to a whole number of 128-row chunks so each loads
    # with a single rearranging DMA
    w2t = nc.dram_tensor("w2t", [KC2 * 128, H2], BF16, kind="ExternalInput")
    w3t = nc.dram_tensor("w3t", [KC3 * 128, A], BF16, kind="ExternalInput")
    # all biases in one [128, MC1+MC2+1] tensor (cols: b1 chunks, b2
    # chunks, b3)
    bb = nc.dram_tensor("bb", [128, MC1 + MC2 + 1], F32, kind="ExternalInput")
    out = nc.dram_tensor("out", [nt * NTILE, A], F32, kind="ExternalOutput")

    with tile.TileContext(nc) as tc:
        with (
            tc.tile_pool(name="consts", bufs=1) as consts,
            tc.tile_pool(name="state", bufs=6) as state_pool,
            tc.tile_pool(name="acts", bufs=2) as acts,
            tc.tile_pool(name="epi", bufs=2) as epi,
            tc.tile_pool(name="ps_x1", bufs=4, space="PSUM") as ps_x1,
            tc.tile_pool(name="ps_x2", bufs=3, space="PSUM") as ps_x2,
            tc.tile_pool(name="ps_y", bufs=1, space="PSUM") as ps_y,
        ):
            # consts go through the ACT-engine HWDGE queue so the SP queue
            # can start streaming state tiles immediately
            w1_sb = consts.tile([128, KC1, H1], BF16)
            nc.scalar.dma_start(out=w1_sb, in_=w1t[:].rearrange("k p c -> p k c"))
            w2_sb = consts.tile([128, KC2, H2], BF16)
            nc.scalar.dma_start(
                out=w2_sb, in_=w2t[:].rearrange("(k p) c -> p k c", p=128)
            )
            # duplicate the ragged K=16 chunk at partition base 32 for the
            # row-packed B-side matmuls
            nc.scalar.dma_start(out=w2_sb[32:48, 3, :], in_=w2t[384:400, :])
            w3_sb = consts.tile([128, KC3, A], BF16)
            nc.scalar.dma_start(
                out=w3_sb, in_=w3t[:].rearrange("(k p) c -> p k c", p=128)
            )
            # duplicate the ragged K=44 chunk at partition base 64
            nc.scalar.dma_start(out=w3_sb[64:108, 2, :], in_=w3t[256:300, :])
            bb_sb = consts.tile([128, MC1 + MC2 + 1], F32)
            nc.scalar.dma_start(out=bb_sb, in_=bb[:])
            b1_sb = bb_sb[:, 0:MC1]
            b2_sb = bb_sb[:, MC1 : MC1 + MC2]
            b3_sb = bb_sb[:, MC1 + MC2 : MC1 + MC2 + 1]

            # Tile inserts a full engine drain at the For_i back-edge
            # (~20us of pipeline refill per iteration). Unroll the body 2x
            # so the drain is paid half as often.
            unroll = 2 if repeat > 1 and repeat % 2 == 0 else 1
            n_iter = repeat // unroll
            rep_ctx = (
                tc.For_i(0, n_iter, 1) if n_iter > 1 else contextlib.nullcontext()
            )
            with rep_ctx:
                for _u in range(unroll):
                    _tile_loop(nc, tc, nt, locals(), cold_start=(_u == 0))

    nc.compile()
    _dedupe_ldweights(nc)
    return nc


def _dedupe_ldweights(nc):
    """Remove back-to-back redundant PE Ldweights (identical stationary AP
    and tile position, no waits) — the paired A/B matmuls share each
    stationary, so every second load is a no-op the sequencer still pays
    for. Transpose matmuls self-load and clobber the array, resetting the
    tracked signature."""
    import bass_rust

    PE = mybir.EngineType.PE
    n_del = 0
    for fn in nc.m.functions:
        for blk in fn.blocks:
            insts = list(blk.instructions)
            last_sig = None
            keep = []
            for ins in insts:
                if getattr(ins, "engine", None) != PE:
                    keep.append(ins)
                    continue
                tn = type(ins).__name__
                if tn == "InstLdweights":
                    try:
                        ap = ins.ins[0]
                        sig = (
                            str(ap.memref),
                            int(ap.offset),
                            str(ap.ap),
                            str(ins.tile_position),
                            str(ins.perf_mode),
                        )
                    except Exception:
                        sig = None
                    if (
                        sig is not None
                        and sig == last_sig
                        and not ins.has_wait()
                        and not ins.has_update()
                    ):
                        n_del += 1
                        continue
                    last_sig = sig
                    keep.append(ins)
                elif tn == "InstMatmult":
                    if ins.is_transpose:
                        last_sig = None
                    keep.append(ins)
                else:
                    keep.append(ins)
            if len(keep) != len(insts):
                blk.instructions = keep
    return n_del


def _tile_loop(nc, tc, nt, env, cold_start=True):
    xt, out = env["xt"], env["out"]
    w1_sb, w2_sb, w3_sb = env["w1_sb"], env["w2_sb"], env["w3_sb"]
    b1_sb, b2_sb, b3_sb = env["b1_sb"], env["b2_sb"], env["b3_sb"]
    state_pool, acts, epi = env["state_pool"], env["acts"], env["epi"]
    ps_x1, ps_x2, ps_y = env["ps_x1"], env["ps_x2"], env["ps_y"]
    assert nt % 2 == 0
    # A/B partition bases for the ragged ("tail") chunks. The B side of the
    # pair runs its tail matmuls in a disjoint PE array quadrant so both
    # tiles' tails execute concurrently (tile_position packing).
    L1_TAIL_B = 32    # L1 m3 (M=16): A cols 0-15, B cols 32-47
    L2_TAIL_B = 64    # L2 m2 (M=44): A cols 0-43, B cols 64-107
    Y_B = 64          # L3 (M=64): A cols 0-63, B cols 64-127
    n_pairs = nt // 2
    GRP = 1   # pairs per epilogue group
    yb_q = None
    pending_epi = None
    for jp in range(n_pairs):
        g = jp % GRP
        pair = (2 * jp, 2 * jp + 1)
        sts = []
        for pi, j in enumerate(pair):
            st = state_pool.tile(
                [128, KC1, NTILE], BF16, tag=f"st{pi}", name=f"st{pi}"
            )
            # at the iteration start (cold refill after the For_i drain)
            # split the two loads across both HWDGE queues; mid-stream,
            # keep everything on SP so prefetch backpressure never blocks
            # the ACT queue
            q = nc.scalar if (cold_start and jp == 0 and pi == 1) else nc.sync
            q.dma_start(out=st, in_=xt[j])
            sts.append(st)

        # Layer 1. Full-width m chunks use one PSUM group per (m, side);
        # the M=16 tail is col-packed: A at cols 0-15, B at cols 32-47 of
        # one shared PSUM tile (concurrent in disjoint array col groups).
        x1s = [
            acts.tile([128, KC2, NTILE], BF16, tag=f"x1{pi}", name=f"x1{pi}")
            for pi in range(2)
        ]
        for m, (mo, mw) in enumerate(CH_H1[:-1]):
            pss = [
                ps_x1.tile([128, NTILE], F32, tag="psx1", name=f"psx1{pi}")
                for pi in range(2)
            ]
            prev = None
            for k in range(KC1):
                for pi in range(2):
                    mm = nc.tensor.matmul(
                        out=pss[pi][0:mw, :],
                        lhsT=w1_sb[:, k, mo : mo + mw],
                        rhs=sts[pi][:, k, :],
                        start=(k == 0),
                        stop=(k == KC1 - 1),
                    )
                    if prev is not None:
                        add_dep_helper(
                            mm.ins, prev.ins, sync=False, reason="pair ldw share"
                        )
                    prev = mm
            for pi in range(2):
                nc.scalar.activation(
                    out=x1s[pi][0:mw, m, :],
                    in_=pss[pi][0:mw, :],
                    func=AF.Relu,
                    bias=b1_sb[0:mw, m : m + 1],
                )
        mo, mw = CH_H1[-1]
        m3 = len(CH_H1) - 1
        bases1 = (0, L1_TAIL_B)
        ps13 = ps_x1.tile([128, NTILE], F32, tag="psx1", name="ps13")
        for k in range(KC1):
            for pi in range(2):
                b = bases1[pi]
                nc.tensor.matmul(
                    out=ps13[b : b + mw, :],
                    lhsT=w1_sb[:, k, mo : mo + mw],
                    rhs=sts[pi][:, k, :],
                    start=(k == 0),
                    stop=(k == KC1 - 1),
                    tile_position=(0, b),
                    skip_group_check=True,
                )
        for pi in range(2):
            b = bases1[pi]
            nc.scalar.activation(
                out=x1s[pi][b : b + mw, m3, :],
                in_=ps13[b : b + mw, :],
                func=AF.Relu,
                bias=b1_sb[b : b + mw, m3 : m3 + 1],
            )

        # Layer 2. x1's K=16 tail lives at base 0 (A) / 32 (B): row-packed,
        # so the pair's ragged-k matmuls run concurrently in disjoint row
        # groups. The M=44 tail is col-packed at base 0 (A) / 64 (B).
        def l2_k_ap(pi, k, weights_m_slice):
            ko, kw = CH_H1[k]
            b = bases1[pi] if k == m3 else 0
            return (
                w2_sb[b : b + kw, k, weights_m_slice],
                x1s[pi][b : b + kw, k, :],
                b,
            )

        x2s = [
            acts.tile([128, KC3, NTILE], BF16, tag=f"x2{pi}", name=f"x2{pi}")
            for pi in range(2)
        ]
        for m, (mo, mw) in enumerate(CH_H2[:-1]):
            pss = [
                ps_x2.tile([128, NTILE], F32, tag="psx2", name=f"psx2{pi}")
                for pi in range(2)
            ]
            prev = None
            for k in range(KC2):
                for pi in range(2):
                    lhsT, rhs, kb = l2_k_ap(pi, k, slice(mo, mo + mw))
                    mm = nc.tensor.matmul(
                        out=pss[pi][0:mw, :],
                        lhsT=lhsT,
                        rhs=rhs,
                        start=(k == 0),
                        stop=(k == KC2 - 1),
                        tile_position=(kb, 0),
                    )
                    if prev is not None and kb == 0:
                        add_dep_helper(
                            mm.ins, prev.ins, sync=False, reason="pair ldw share"
                        )
                    prev = mm
            for pi in range(2):
                nc.vector.tensor_scalar(
                    out=x2s[pi][0:mw, m, :],
                    in0=pss[pi][0:mw, :],
                    scalar1=b2_sb[0:mw, m : m + 1],
                    scalar2=0.0,
                    op0=OP.add,
                    op1=OP.max,
                )
        mo, mw = CH_H2[-1]
        m2 = len(CH_H2) - 1
        bases2 = (0, L2_TAIL_B)
        ps22 = ps_x2.tile([128, NTILE], F32, tag="psx2", name="ps22")
        for k in range(KC2):
            for pi in range(2):
                lhsT, rhs, kb = l2_k_ap(pi, k, slice(mo, mo + mw))
                b = bases2[pi]
                nc.tensor.matmul(
                    out=ps22[b : b + mw, :],
                    lhsT=lhsT,
                    rhs=rhs,
                    start=(k == 0),
                    stop=(k == KC2 - 1),
                    tile_position=(kb, b),
                    skip_group_check=True,
                )
        for pi in range(2):
            b = bases2[pi]
            nc.vector.tensor_scalar(
                out=x2s[pi][b : b + mw, m2, :],
                in0=ps22[b : b + mw, :],
                scalar1=b2_sb[b : b + mw, m2 : m2 + 1],
                scalar2=0.0,
                op0=OP.add,
                op1=OP.max,
            )

        # Layer 3: both sides col-packed into one PSUM bank
        # (A rows 0-63, B rows 64-127).
        y_ps = ps_y.tile([128, NTILE], F32, tag="yy", name="psy")
        for k, (ko, kw) in enumerate(CH_H2):
            for pi in range(2):
                kb = bases2[pi] if k == m2 else 0
                yb_base = (0, Y_B)[pi]
                nc.tensor.matmul(
                    out=y_ps[yb_base : yb_base + A, :],
                    lhsT=w3_sb[kb : kb + kw, k, :],
                    rhs=x2s[pi][kb : kb + kw, k, :],
                    start=(k == 0),
                    stop=(k == KC3 - 1),
                    tile_position=(kb, yb_base),
                    skip_group_check=True,
                )
        if g == 0:
            yb_q = epi.tile([128, GRP, NTILE], BF16, tag="yb", name="yb")
        nc.scalar.activation(
            out=yb_q[:, g, :], in_=y_ps, func=AF.Identity, bias=b3_sb
        )
        if g != GRP - 1:
            continue

        # One DMA-XBAR transpose flips the whole group's y back to batch-
        # major: block index 4*g+j2 of yt_q holds pair g's sub-tile j2,
        # with side A's features in cols 0-63 and B's in 64-127. Frees
        # the PE of transpose passes and amortizes the DMA instruction.
        yt_q = epi.tile([128, GRP * SUB, 128], BF16, tag="yt", name="yt")
        nc.scalar.dma_start_transpose(out=yt_q, in_=yb_q)

        # Software-pipeline the post-transpose epilogue by ONE pair: the
        # tanh of pair p issues only during pair p+1, when its transpose
        # has long completed — otherwise the tanh's wait head-of-line
        # blocks the next pair's L1 PSUM evacuations on the ACT queue.
        if pending_epi is not None:
            pending_epi()
        pending_epi = lambda yt_q=yt_q, jp=jp: _post_transpose_epi(
            nc, env, yt_q, jp
        )
    pending_epi()


def _post_transpose_epi(nc, env, yt_q, jp):
    out, epi = env["out"], env["epi"]
    GRP = 1
    if True:
        # Epilogue: tanh head, sigmoid last column, positive-sum
        # normalization with per-partition (per-row) scalars.
        out_q = epi.tile([128, 2 * GRP, SUB, A], F32, tag="outp", name="outp")
        for gg in range(GRP):
            yt_sb = yt_q[:, gg * SUB : (gg + 1) * SUB, :]
            s8 = epi.tile([128, 2 * SUB], F32, tag=f"s8{gg}", name=f"s8{gg}")
            rinv8 = epi.tile(
                [128, 2 * SUB], F32, tag=f"rinv8{gg}", name=f"rinv8{gg}"
            )
            # one tanh over both sides' heads: view yt as [p, SUB, 2, 64]
            # (side t at cols 64t..64t+63), take cols 0..62 of each side
            yt4 = yt_sb.rearrange("p s (t c) -> p s t c", t=2)
            tanh_q = epi.tile(
                [128, SUB, 2, A - 1], F32, tag=f"tanh{gg}", name=f"tanh{gg}"
            )
            nc.scalar.activation(
                out=tanh_q, in_=yt4[:, :, :, 0 : A - 1], func=AF.Tanh
            )
            relu_q = epi.tile(
                [128, SUB, 2, A - 1], F32, tag=f"reluh{gg}", name=f"reluh{gg}"
            )
            for pi in range(2):
                for j2 in range(SUB):
                    idx = pi * SUB + j2
                    nc.vector.tensor_scalar(
                        out=relu_q[:, j2, pi, :],
                        in0=tanh_q[:, j2, pi, :],
                        scalar1=0.0,
                        scalar2=None,
                        op0=OP.max,
                        op1=OP.add,
                        accum_out=s8[:, idx : idx + 1],
                    )
            nc.vector.tensor_scalar_max(s8, s8, 1e-30)
            nc.vector.reciprocal(rinv8, s8)
            nc.vector.tensor_scalar_add(rinv8, rinv8, -1.0)
            # one sigmoid covers both sides' last column
            nc.scalar.activation(
                out=out_q[:, 2 * gg : 2 * gg + 2, :, A - 1 : A],
                in_=yt4[:, :, :, A - 1 : A].rearrange("p s t c -> p t s c"),
                func=AF.Sigmoid,
            )
            for pi in range(2):
                for j2 in range(SUB):
                    idx = pi * SUB + j2
                    nc.vector.scalar_tensor_tensor(
                        out=out_q[:, 2 * gg + pi, j2, 0 : A - 1],
                        in0=relu_q[:, j2, pi, :],
                        scalar=rinv8[:, idx : idx + 1],
                        in1=tanh_q[:, j2, pi, :],
                        op0=OP.mult,
                        op1=OP.add,
                    )
        # host packs tile column n=128*j2+p as original row 4p+j2, so
        # partition p's 4 result rows per tile are DRAM-contiguous (1KB);
        # the whole group's tiles store with ONE DMA on the idle GpSimd
        # SWDGE queue to keep HWDGE free
        t0 = (2 * (jp - GRP + 1)) * NTILE
        nc.gpsimd.dma_start(
            out=out[t0 : t0 + 2 * GRP * NTILE, :].rearrange(
                "(t p q) c -> p t q c", t=2 * GRP, p=128
            ),
            in_=out_q,
        )


def prep_core_inputs(state_c, w1, b1, w2, b2, w3, b3, nt=NT):
    """Build the per-core input map from this core's [BC, 512] f32 shard."""
    bf16 = ml_dtypes.bfloat16
    rows = nt * NTILE

    def pad_bias(b, n_chunks, tail_dup_base=None):
        # [n] -> [128, n_chunks] column-per-chunk layout (ragged tail zero);
        # optionally duplicate the ragged tail chunk at a shifted partition
        # base for the B-side packed matmuls
        out = np.zeros((128, n_chunks), np.float32)
        cs = chunks(len(b))
        for m, (mo, mw) in enumerate(cs):
            out[0:mw, m] = b[mo : mo + mw]
        if tail_dup_base is not None:
            mo, mw = cs[-1]
            out[tail_dup_base : tail_dup_base + mw, -1] = b[mo : mo + mw]
        return out

    # xt[j, p, k, n] = state_c[NTILE*j + perm[n], 128*k + p] — contiguous
    # per partition so each tile loads with one DMA. perm places original
    # row 4p+j2 at column n=128*j2+p, which makes each partition's 4
    # output rows DRAM-contiguous after the epilogue's XBAR transpose.
    n = np.arange(NTILE)
    perm = 4 * (n % 128) + n // 128
    xt = np.ascontiguousarray(
        state_c[:rows]
        .reshape(nt, NTILE, KC1, 128)[:, perm]
        .transpose(0, 3, 2, 1)
    ).astype(bf16)

    def pad_rows(w, rows128):
        out = np.zeros((rows128 * 128, w.shape[1]), w.dtype)
        out[: w.shape[0]] = w
        return out

    bb = np.concatenate(
        [
            pad_bias(b1, MC1, tail_dup_base=32),
            pad_bias(b2, MC2, tail_dup_base=64),
            np.concatenate([b3, b3]).reshape(128, 1).astype(np.float32),
        ],
        axis=1,
    )
    return {
        "xt": xt,
        "w1t": np.ascontiguousarray(w1.T.reshape(KC1, 128, H1)).astype(bf16),
        "w2t": np.ascontiguousarray(pad_rows(w2.T, KC2)).astype(bf16),
        "w3t": np.ascontiguousarray(pad_rows(w3.T, KC3)).astype(bf16),
        "bb": np.ascontiguousarray(bb).astype(np.float32),
    }


_NC_CACHE = {}


def _get_nc(nt=NT):
    if nt not in _NC_CACHE:
        _NC_CACHE[nt] = build_kernel(nt)
    return _NC_CACHE[nt]


def run(state, W1, b1, W2, b2, W3, b3, trace=False, **spmd_kwargs):
    state = np.asarray(state, dtype=np.float32)
    nc = _get_nc()
    in_maps = []
    for c in range(N_CORES):
        shard = state[c * BC : (c + 1) * BC]
        in_maps.append(prep_core_inputs(shard, W1, b1, W2, b2, W3, b3))
    res = run_bass_kernel_spmd(
        nc, in_maps, core_ids=list(range(N_CORES)), trace=trace, **spmd_kwargs
    )
    full = np.concatenate([np.asarray(r["out"]) for r in res.results], axis=0)
    return full.astype(np.float32), res


def kernel(state, W1, b1, W2, b2, W3, b3):
    out, _ = run(state, W1, b1, W2, b2, W3, b3, trace=False)
    return out


def make_runner(nc, in_maps):
    """Build a reusable on-device executor with resident inputs.

    Mirrors bass2jax.run_bass_via_pjrt's multi-core path but without
    donation (this kernel writes every output element) so device inputs
    can be reused across calls — per-call cost is dispatch + execution.
    """
    import jax
    from jax.experimental.shard_map import shard_map
    from jax.sharding import Mesh, NamedSharding, PartitionSpec

    from concourse.bass2jax import (
        _bass_exec_p,
        install_neuronx_cc_hook,
        partition_id_tensor,
    )

    install_neuronx_cc_hook()
    n_cores = len(in_maps)
    partition_name = nc.partition_id_tensor.name if nc.partition_id_tensor else None
    in_names, out_names, out_avals, zero_outs = [], [], [], []
    for alloc in nc.m.functions[0].allocations:
        if not isinstance(alloc, mybir.MemoryLocationSet):
            continue
        name = alloc.memorylocations[0].name
        if alloc.kind == "ExternalInput":
            if name != partition_name:
                in_names.append(name)
        elif alloc.kind == "ExternalOutput":
            shape = tuple(alloc.tensor_shape)
            dtype = mybir.dt.np(alloc.dtype)
            out_names.append(name)
            out_avals.append(jax.core.ShapedArray(shape, dtype))
            zero_outs.append(np.zeros(shape, dtype))
    n_params = len(in_names)
    all_in = list(in_names) + list(out_names)
    if partition_name is not None:
        all_in.append(partition_name)

    def _body(*args):
        operands = list(args)
        if partition_name is not None:
            operands.append(partition_id_tensor())
        outs = _bass_exec_p.bind(
            *operands,
            out_avals=tuple(out_avals),
            in_names=tuple(all_in),
            out_names=tuple(out_names),
            lowering_input_output_aliases=(),
            sim_require_finite=True,
            sim_require_nnan=True,
            nc=nc,
        )
        return tuple(outs)

    devices = jax.devices()[:n_cores]
    mesh = Mesh(np.asarray(devices), ("core",))
    in_specs = (PartitionSpec("core"),) * (n_params + len(out_names))
    out_specs = (PartitionSpec("core"),) * len(out_names)
    fn = jax.jit(
        shard_map(
            _body, mesh=mesh, in_specs=in_specs, out_specs=out_specs, check_rep=False
        ),
        keep_unused=True,
    )
    sharding = NamedSharding(mesh, PartitionSpec("core"))
    concat_in = [
        np.concatenate([np.asarray(in_maps[c][nm]) for c in range(n_cores)], axis=0)
        for nm in in_names
    ]
    concat_zero = [
        np.zeros((n_cores * z.shape[0], *z.shape[1:]), z.dtype) for z in zero_outs
    ]
    dev_args = [jax.device_put(a, sharding) for a in concat_in + concat_zero]

    def call():
        outs = fn(*dev_args)
        jax.block_until_ready(outs)
        return outs

    call.fn = fn
    call.dev_args = dev_args
    return call, out_names, out_avals
